# revision 1
# baseline (speedup 1.0000x reference)
"""Contrastive loss (GRACE-style) on 8 Trainium2 NeuronCores — fp8 edition.

loss = sum_i 0.5*(l1_i + l2_i)
  l1 = -log(diag(exp(h1@h2.T/t)) / (rowsum(exp(h1@h1.T/t)) + rowsum(exp(h1@h2.T/t)) - diag(exp(h1@h1.T/t))))
  l2 = same with h1<->h2;  h = z / ||z||_row,  t = 0.2

Sharding: columns (j) of the similarity matrices are sharded across 8 cores
(each core owns a 1024-column chunk of both h1 and h2). Each core computes,
for ALL 8192 rows i, the partial sums over its j-chunk of
exp(s_i * (z_i . h_j)), where the row normalization 1/(16*tau*||z_i||) is a
per-partition ACT scale. All matmuls run fp8(e4m3) DoubleRow (2x PE rate):
stationary = raw z.T tiles, moving = (h*16).T chunk tiles, contraction pairs
along the kd dimension. Per row-block: exp+rowsum of refl and between tiles
(ACT exp -> DVE reduce), plus ones-matmul colsums of exp'd between tiles
(= between.T rowsums for own chunk rows, PSUM-accumulated over all 64 row
blocks). Host (numpy, O(N*D)) prepares fp8 inputs/scales and applies the
exact diagonal corrections + logs in float64.
"""

import numpy as np
import ml_dtypes

N = 8192
D = 512
NCORES = 8
CH = N // NCORES  # 1024 columns per core
P = 128
KD = D // P  # 4 k-subtiles; DoubleRow consumes them in pairs
NIB = N // P  # 64 row blocks
TAU = 0.2
RS = 16.0  # rhs pre-scale to keep fp8 h values in the normal range

_CACHE = {}


def _build(repeat=1, loop=None):
    import concourse.tile as tile
    from concourse import bacc, mybir

    f32 = mybir.dt.float32
    bf16 = mybir.dt.bfloat16
    fp8 = mybir.dt.float8e4
    AF = mybir.ActivationFunctionType
    ALU = mybir.AluOpType
    DR = mybir.MatmulPerfMode.DoubleRow

    nc = bacc.Bacc("TRN2", target_bir_lowering=False, debug=False,
                   num_devices=NCORES)

    zt1 = nc.dram_tensor("zt1", [D, N], fp8, kind="ExternalInput")
    zt2 = nc.dram_tensor("zt2", [D, N], fp8, kind="ExternalInput")
    rh1 = nc.dram_tensor("rh1", [D, CH], fp8, kind="ExternalInput")
    rh2 = nc.dram_tensor("rh2", [D, CH], fp8, kind="ExternalInput")
    s1 = nc.dram_tensor("s1", [N], f32, kind="ExternalInput")
    s2 = nc.dram_tensor("s2", [N], f32, kind="ExternalInput")
    partials = nc.dram_tensor("partials", [2, N], f32, kind="ExternalOutput")
    ecol = nc.dram_tensor("ecol", [CH], f32, kind="ExternalOutput")

    zt1v = zt1.rearrange("(k p) n -> p k n", p=P)
    zt2v = zt2.rearrange("(k p) n -> p k n", p=P)
    rh1v = rh1.rearrange("(k p) n -> p k n", p=P)
    rh2v = rh2.rearrange("(k p) n -> p k n", p=P)

    with tile.TileContext(nc) as tc:
        with (
            tc.tile_pool(name="singles", bufs=1) as singles,
            tc.tile_pool(name="es", bufs=2) as esp,
            tc.tile_pool(name="ps", bufs=3, space="PSUM") as psp,
            tc.tile_pool(name="pscol", bufs=1, space="PSUM") as pscolp,
        ):
            # ---- persistent buffers ----
            zt1s = singles.tile([P, KD, N], fp8, tag="zt1s")
            zt2s = singles.tile([P, KD, N], fp8, tag="zt2s")
            rh1s = singles.tile([P, KD, CH], fp8, tag="rh1s")
            rh2s = singles.tile([P, KD, CH], fp8, tag="rh2s")
            s1s = singles.tile([P, NIB], f32, tag="s1s")
            s2s = singles.tile([P, NIB], f32, tag="s2s")
            acc1 = singles.tile([P, NIB], f32, tag="acc1")
            acc2 = singles.tile([P, NIB], f32, tag="acc2")
            ones = singles.tile([P, 1], bf16, tag="ones")
            nc.vector.memset(ones, 1.0)
            ecol_s = singles.tile([1, CH], f32, tag="ecol_s")
            cs = pscolp.tile([1, CH], f32, tag="cs")

            # ---- input DMAs (rhs + scales first: needed by every ib) ----
            nc.sync.dma_start(out=rh1s, in_=rh1v)
            nc.sync.dma_start(out=rh2s, in_=rh2v)
            nc.sync.dma_start(out=s1s, in_=s1.rearrange("(b p) -> p b", p=P))
            nc.sync.dma_start(out=s2s, in_=s2.rearrange("(b p) -> p b", p=P))
            nc.sync.dma_start(out=zt1s, in_=zt1v)
            nc.sync.dma_start(out=zt2s, in_=zt2v)

            # ---- main ----
            def _main_body():
                esb_pend = []

                def _colsum(item):
                    ibx, esb = item
                    for jt in range(2):
                        nc.tensor.matmul(
                            cs[0:1, jt * 512:(jt + 1) * 512],
                            lhsT=ones,
                            rhs=esb[:, jt * 512:(jt + 1) * 512],
                            start=(ibx == 0),
                            stop=(ibx == NIB - 1),
                            skip_group_check=True,
                        )

                # pass 1: z1 row blocks x [h1c | h2c] -> refl1 + between1
                for ib in range(NIB):
                    psa = psp.tile([P, 1024], f32, tag="ps", name="psa")
                    psb = psp.tile([P, 1024], f32, tag="ps", name="psb")
                    for kp in range(2):
                        lhsT = zt1s[:, 2 * kp:2 * kp + 2, ib * P:(ib + 1) * P]
                        for jt in range(2):
                            nc.tensor.matmul(
                                psa[:, jt * 512:(jt + 1) * 512],
                                lhsT=lhsT,
                                rhs=rh1s[:, 2 * kp:2 * kp + 2,
                                         jt * 512:(jt + 1) * 512],
                                start=(kp == 0), stop=(kp == 1),
                                perf_mode=DR)
                        for jt in range(2):
                            nc.tensor.matmul(
                                psb[:, jt * 512:(jt + 1) * 512],
                                lhsT=lhsT,
                                rhs=rh2s[:, 2 * kp:2 * kp + 2,
                                         jt * 512:(jt + 1) * 512],
                                start=(kp == 0), stop=(kp == 1),
                                perf_mode=DR)
                    esa = esp.tile([P, 1024], bf16, tag="esa", bufs=3)
                    nc.scalar.activation(out=esa, in_=psa, func=AF.Exp,
                                         scale=s1s[:, ib:ib + 1])
                    esb = esp.tile([P, 1024], bf16, tag="esb", bufs=4)
                    nc.scalar.activation(out=esb, in_=psb, func=AF.Exp,
                                         scale=s1s[:, ib:ib + 1])
                    # refl1+between1 rowsums: 2x-rate add, then one 1x reduce
                    esum = esp.tile([P, 1024], bf16, tag="esum", bufs=3)
                    nc.vector.tensor_add(esum, esa, esb)
                    nc.vector.tensor_reduce(acc1[:, ib:ib + 1], esum,
                                            axis=mybir.AxisListType.X,
                                            op=ALU.add)
                    esb_pend.append((ib, esb))
                    # colsum matmuls lag 2 iterations so PE never waits on ACT
                    if len(esb_pend) > 2:
                        _colsum(esb_pend.pop(0))
                for item in esb_pend:
                    _colsum(item)

                # pass 2: z2 row blocks x h2c -> refl2
                for ib in range(NIB):
                    psc = psp.tile([P, 1024], f32, tag="ps", name="psc")
                    for kp in range(2):
                        lhsT = zt2s[:, 2 * kp:2 * kp + 2, ib * P:(ib + 1) * P]
                        for jt in range(2):
                            nc.tensor.matmul(
                                psc[:, jt * 512:(jt + 1) * 512],
                                lhsT=lhsT,
                                rhs=rh2s[:, 2 * kp:2 * kp + 2,
                                         jt * 512:(jt + 1) * 512],
                                start=(kp == 0), stop=(kp == 1),
                                perf_mode=DR)
                    esc = esp.tile([P, 1024], bf16, tag="esa", bufs=3)
                    nc.scalar.activation(out=esc, in_=psc, func=AF.Exp,
                                         scale=s2s[:, ib:ib + 1])
                    nc.vector.tensor_reduce(acc2[:, ib:ib + 1], esc,
                                            axis=mybir.AxisListType.X,
                                            op=ALU.add)

            if loop is not None:
                with tc.For_i(0, loop):
                    _main_body()
            else:
                for _rep in range(repeat):
                    _main_body()

            # stage colsums to SBUF
            nc.vector.tensor_copy(ecol_s, cs)
            nc.sync.dma_start(out=ecol[:].rearrange("(o c) -> o c", o=1),
                              in_=ecol_s)
            nc.sync.dma_start(
                out=partials[0].rearrange("(b p) -> p b", p=P), in_=acc1)
            nc.sync.dma_start(
                out=partials[1].rearrange("(b p) -> p b", p=P), in_=acc2)

    nc.compile()
    return nc


def _get_nc(repeat=1, loop=None):
    key = ("nc", repeat, loop)
    if key not in _CACHE:
        _CACHE[key] = _build(repeat, loop=loop)
    return _CACHE[key]


def _host_prep(z1, z2):
    fp8 = ml_dtypes.float8_e4m3
    z1 = np.asarray(z1, dtype=np.float32)
    z2 = np.asarray(z2, dtype=np.float32)
    n1 = np.maximum(np.linalg.norm(z1, axis=1), 1e-12)
    n2 = np.maximum(np.linalg.norm(z2, axis=1), 1e-12)
    h1 = z1 / n1[:, None]
    h2 = z2 / n2[:, None]
    z1_8 = z1.astype(fp8)
    z2_8 = z2.astype(fp8)
    r1_8 = (h1 * RS).astype(fp8)
    r2_8 = (h2 * RS).astype(fp8)
    s1 = (1.0 / (RS * TAU * n1)).astype(np.float32)
    s2 = (1.0 / (RS * TAU * n2)).astype(np.float32)
    return z1_8, z2_8, r1_8, r2_8, s1, s2, h1, h2, n1, n2


def make_in_maps(z1, z2):
    z1_8, z2_8, r1_8, r2_8, s1, s2, _, _, _, _ = _host_prep(z1, z2)
    zt1 = np.ascontiguousarray(z1_8.T)
    zt2 = np.ascontiguousarray(z2_8.T)
    rt1 = r1_8.T
    rt2 = r2_8.T
    in_maps = []
    for r in range(NCORES):
        in_maps.append({
            "zt1": zt1, "zt2": zt2,
            "rh1": np.ascontiguousarray(rt1[:, r * CH:(r + 1) * CH]),
            "rh2": np.ascontiguousarray(rt2[:, r * CH:(r + 1) * CH]),
            "s1": s1, "s2": s2,
        })
    return in_maps


def kernel(z1, z2):
    from concourse.bass_utils import run_bass_kernel_spmd

    z1_8, z2_8, r1_8, r2_8, s1, s2, h1, h2, n1, n2 = _host_prep(z1, z2)
    in_maps = make_in_maps(z1, z2)

    nc = _get_nc()
    res = run_bass_kernel_spmd(nc, in_maps, core_ids=list(range(NCORES)))

    S1 = np.zeros(N, dtype=np.float64)
    S2 = np.zeros(N, dtype=np.float64)
    for r in range(NCORES):
        out = res.results[r]
        S1 += out["partials"][0].astype(np.float64)
        S2 += out["partials"][1].astype(np.float64)
        S2[r * CH:(r + 1) * CH] += out["ecol"].astype(np.float64)

    # exact diagonal corrections, computed from the same fp8 data the
    # device used: refl_ii = exp(s_i * (z8_i . r8_i))
    q1 = (z1_8.astype(np.float64) * r1_8.astype(np.float64)).sum(1) \
        * s1.astype(np.float64)
    q2 = (z2_8.astype(np.float64) * r2_8.astype(np.float64)).sum(1) \
        * s2.astype(np.float64)
    v5 = (h1.astype(np.float64) * h2.astype(np.float64)).sum(1) / TAU

    loss = 0.5 * (np.log(S1 - np.exp(q1)) + np.log(S2 - np.exp(q2))) - v5
    return np.float32(loss.sum())



# revision 8
# speedup vs baseline: 1.0240x; 1.0240x over previous
"""Contrastive loss (GRACE-style) on 8 Trainium2 NeuronCores — symmetric fp8.

loss = sum_i 0.5*(l1_i + l2_i)
  l1 = log(rowsum(exp(h1@h1.T/t)) + rowsum(exp(h1@h2.T/t)) - diag_refl) - log(diag_bet)
  l2 = same with h1<->h2;  h = z / ||z||_row,  t = 0.2

Device computes, in fp8 DoubleRow with a single array rh = fp8(16*h) serving
both matmul operands (exp scale c = 1/(256*t) is a constant immediate):
  - refl1/refl2 exploiting symmetry: row block a computes the contiguous
    (mod N) column window [128a, 128a + W), W = 33 blocks for a<32 else 32.
    Row sums come free via ACT accum_out; the transposed halves are credited
    by elementwise-accumulating exp tiles into colacc (DVE) and partition-
    reducing at the end (GPSIMD axis=C). Every off-diag window block is
    credited; the diag block is included in the window but not credited.
  - between1: row-sharded, each unit computes all N columns. Row sums via
    accum_out; column sums (= between2 row sums, exact transpose since the
    same fp8 operand pair feeds both) via bcolacc + partition-reduce.
Work is sharded round-robin: core r owns row blocks a ≡ r (mod 8). Each core
receives its input pre-rotated by 128r columns (and extended by 4224 wrap
columns), so all cores execute the identical SPMD program; the host un-rotates
the column-indexed outputs. Host (numpy, O(N*D)) applies exact diagonal
corrections + logs in float64.
"""

import numpy as np
import ml_dtypes

N = 8192
D = 512
NCORES = 8
P = 128
KD = D // P          # 4 k-subtiles; DoubleRow consumes them in pairs
NB = N // P          # 64 row blocks
UPC = NB // NCORES   # 8 units per core per matrix
TAU = 0.2
RS = 16.0            # fp8 h pre-scale; keeps values in the normal range
CEXP = 1.0 / (RS * RS * TAU)
WEXT = 33 * P        # wrap extension: widest window
WN = N + WEXT        # per-core input width (rotated + extended)

# unit u handles row block a = r + 8u; a<32 <=> u<4 (window incl dist-32 tail)
UNIT_CHUNKS = [(1536, 1536, 1152)] * 4 + [(2048, 2048)] * 4
BCH = 2048           # between-pass chunk width

_CACHE = {}


def _fold_spans(start, end):
    """Split local span [start, end) at the N wrap.

    Returns (fold_start, fold_end, off) pieces where off is the piece's
    offset from the span start in the unfolded (source buffer) coordinates.
    """
    out = []
    if start < N:
        out.append((start, min(end, N), 0))
    if end > N:
        s = max(start, N)
        out.append((s - N, end - N, s - start))
    return out


def _build(repeat=1, loop=None):
    import concourse.tile as tile
    from concourse import bacc, mybir

    f32 = mybir.dt.float32
    bf16 = mybir.dt.bfloat16
    fp8 = mybir.dt.float8e4
    AF = mybir.ActivationFunctionType
    ALU = mybir.AluOpType
    AX = mybir.AxisListType
    DR = mybir.MatmulPerfMode.DoubleRow

    nc = bacc.Bacc("TRN2", target_bir_lowering=False, debug=False,
                   num_devices=NCORES)

    rhw1 = nc.dram_tensor("rhw1", [D, WN], fp8, kind="ExternalInput")
    rhw2 = nc.dram_tensor("rhw2", [D, WN], fp8, kind="ExternalInput")
    racc1 = nc.dram_tensor("racc1", [P, 3 * UPC], f32, kind="ExternalOutput")
    racc2 = nc.dram_tensor("racc2", [P, 3 * UPC], f32, kind="ExternalOutput")
    bacc_d = nc.dram_tensor("bacc", [P, 4 * UPC], f32, kind="ExternalOutput")
    ccol1 = nc.dram_tensor("ccol1", [N], f32, kind="ExternalOutput")
    ccol2 = nc.dram_tensor("ccol2", [N], f32, kind="ExternalOutput")
    bcol = nc.dram_tensor("bcol", [N], f32, kind="ExternalOutput")

    rhw1v = rhw1.rearrange("(k p) n -> p k n", p=P)
    rhw2v = rhw2.rearrange("(k p) n -> p k n", p=P)

    with tile.TileContext(nc) as tc:
        with (
            tc.tile_pool(name="singles", bufs=1) as singles,
            tc.tile_pool(name="es", bufs=4) as esp,
            tc.tile_pool(name="st", bufs=2) as stp,
            tc.tile_pool(name="ps", bufs=2, space="PSUM") as psp,
        ):
            rh1s = singles.tile([P, KD, WN], fp8, tag="rh1s")
            rh2s = singles.tile([P, KD, WN], fp8, tag="rh2s")
            colacc = singles.tile([P, N], bf16, tag="colacc")
            bcolacc = singles.tile([P, N], bf16, tag="bcolacc")
            racc1s = singles.tile([P, 3 * UPC], f32, tag="racc1s")
            racc2s = singles.tile([P, 3 * UPC], f32, tag="racc2s")
            baccs = singles.tile([P, 4 * UPC], f32, tag="baccs")
            nc.vector.memset(racc1s, 0.0)
            nc.vector.memset(racc2s, 0.0)

            # input DMAs, sliced so compute starts after the first pieces land
            NPC = 7  # 6x2048 + 128
            for t, (sb, dv) in enumerate(((rh1s, rhw1v), (rh2s, rhw2v))):
                for i in range(NPC):
                    a, b = i * 2048, min((i + 1) * 2048, WN)
                    nc.sync.dma_start(out=sb[:, :, a:b], in_=dv[:, :, a:b])

            def _mm_chunk(rhs_src, lhs_src, u, ps, c0, csize, base):
                lhsT0 = 1024 * u
                for kp in range(2):
                    lhsT = lhs_src[:, 2 * kp:2 * kp + 2, lhsT0:lhsT0 + P]
                    for off in range(0, csize, 512):
                        w = min(512, csize - off)
                        s = base + c0 + off
                        nc.tensor.matmul(
                            ps[:, off:off + w], lhsT=lhsT,
                            rhs=rhs_src[:, 2 * kp:2 * kp + 2, s:s + w],
                            start=(kp == 0), stop=(kp == 1), perf_mode=DR)

            def _credit(es, spans, touched):
                # spans: list of (fold_start, fold_end, es_off); emit runs of
                # copy (first touch) / add per 128-col block state
                for fs, fe, eo in spans:
                    b0, b1 = fs // P, fe // P
                    b = b0
                    while b < b1:
                        state = touched[b]
                        e = b
                        while e < b1 and touched[e] == state:
                            touched[e] = True
                            e += 1
                        dst = colacc[:, b * P:e * P]
                        src = es[:, eo + (b - b0) * P: eo + (e - b0) * P]
                        if state:
                            nc.vector.tensor_add(dst, dst, src)
                        else:
                            nc.vector.tensor_copy(dst, src)
                        b = e

            def _creduce(src, dst_dram, phase_tag):
                from concourse import bass_isa

                for i in range(4):
                    a, b = i * 2048, (i + 1) * 2048
                    st = stp.tile([P, 2048], f32, tag="st",
                                  name=f"st_{phase_tag}_{i}")
                    nc.gpsimd.partition_all_reduce(
                        st, src[:, a:b], P, bass_isa.ReduceOp.add)
                    nc.sync.dma_start(
                        out=dst_dram.rearrange("(o c) -> o c", o=1)[:, a:b],
                        in_=st[0:1, :])

            def _refl_phase(rhs_s, raccs, dst_dram):
                touched = [False] * NB
                for u in range(UPC):
                    chunks = UNIT_CHUNKS[u]
                    c0 = 0
                    for ci, csize in enumerate(chunks):
                        ps = psp.tile([P, 2048], f32, tag="ps", name="psr")
                        _mm_chunk(rhs_s, rhs_s, u, ps, c0, csize, 1024 * u)
                        es = esp.tile([P, 2048], bf16, tag="es", name="esr")
                        nc.scalar.activation(
                            out=es[:, :csize], in_=ps[:, :csize], func=AF.Exp,
                            scale=CEXP,
                            accum_out=raccs[:, 3 * u + ci:3 * u + ci + 1])
                        lo = max(c0, P)  # exclude diag block from credits
                        hi = c0 + csize
                        if lo < hi:
                            spans = [
                                (fs, fe, lo - c0 + eo)
                                for fs, fe, eo in _fold_spans(
                                    1024 * u + lo, 1024 * u + hi)]
                            _credit(es, spans, touched)
                        c0 += csize
                _creduce(colacc, dst_dram, dst_dram.name)

            def _between_phase():
                for u in range(UPC):
                    for ci in range(N // BCH):
                        c0 = ci * BCH
                        ps = psp.tile([P, 2048], f32, tag="ps", name="psb")
                        _mm_chunk(rh2s, rh1s, u, ps, 0, BCH, c0)
                        es = esp.tile([P, 2048], bf16, tag="es", name="esb")
                        nc.scalar.activation(
                            out=es, in_=ps, func=AF.Exp, scale=CEXP,
                            accum_out=baccs[:, 4 * u + ci:4 * u + ci + 1])
                        dst = bcolacc[:, c0:c0 + BCH]
                        if u == 0:
                            nc.vector.tensor_copy(dst, es)
                        else:
                            nc.vector.tensor_add(dst, dst, es)
                _creduce(bcolacc, bcol, "bcol")

            def _main_body():
                _refl_phase(rh1s, racc1s, ccol1)
                _between_phase()
                _refl_phase(rh2s, racc2s, ccol2)
                nc.sync.dma_start(out=racc1[:, :], in_=racc1s)
                nc.sync.dma_start(out=racc2[:, :], in_=racc2s)
                nc.sync.dma_start(out=bacc_d[:, :], in_=baccs)

            if loop is not None:
                with tc.For_i(0, loop):
                    _main_body()
            else:
                for _rep in range(repeat):
                    _main_body()

    nc.compile()
    return nc


def _get_nc(repeat=1, loop=None):
    key = ("nc", repeat, loop)
    if key not in _CACHE:
        _CACHE[key] = _build(repeat, loop=loop)
    return _CACHE[key]


def _host_prep(z1, z2):
    fp8 = ml_dtypes.float8_e4m3
    z1 = np.asarray(z1, dtype=np.float32)
    z2 = np.asarray(z2, dtype=np.float32)
    n1 = np.maximum(np.linalg.norm(z1, axis=1), 1e-12)
    n2 = np.maximum(np.linalg.norm(z2, axis=1), 1e-12)
    h1 = z1 / n1[:, None]
    h2 = z2 / n2[:, None]
    r1_8 = (h1 * RS).astype(fp8)
    r2_8 = (h2 * RS).astype(fp8)
    return r1_8, r2_8, h1, h2


def make_in_maps(z1, z2):
    r1_8, r2_8, _, _ = _host_prep(z1, z2)
    d1 = np.concatenate([r1_8.T, r1_8.T], axis=1)
    d2 = np.concatenate([r2_8.T, r2_8.T], axis=1)
    in_maps = []
    for r in range(NCORES):
        o = P * r
        in_maps.append({
            "rhw1": np.ascontiguousarray(d1[:, o:o + WN]),
            "rhw2": np.ascontiguousarray(d2[:, o:o + WN]),
        })
    return in_maps


def kernel(z1, z2):
    from concourse.bass_utils import run_bass_kernel_spmd

    r1_8, r2_8, h1, h2 = _host_prep(z1, z2)
    in_maps = make_in_maps(z1, z2)

    nc = _get_nc()
    res = run_bass_kernel_spmd(nc, in_maps, core_ids=list(range(NCORES)))

    S1 = np.zeros((NB, P), dtype=np.float64)
    S2 = np.zeros((NB, P), dtype=np.float64)
    Sb1 = np.zeros((NB, P), dtype=np.float64)
    C1 = np.zeros(N, dtype=np.float64)
    C2 = np.zeros(N, dtype=np.float64)
    Sb2 = np.zeros(N, dtype=np.float64)
    for r in range(NCORES):
        out = res.results[r]
        ra1 = out["racc1"].astype(np.float64).reshape(P, UPC, 3).sum(-1)
        ra2 = out["racc2"].astype(np.float64).reshape(P, UPC, 3).sum(-1)
        ba = out["bacc"].astype(np.float64).reshape(P, UPC, 4).sum(-1)
        for u in range(UPC):
            a = r + NCORES * u
            S1[a] += ra1[:, u]
            S2[a] += ra2[:, u]
            Sb1[a] += ba[:, u]
        C1 += np.roll(out["ccol1"].astype(np.float64), P * r)
        C2 += np.roll(out["ccol2"].astype(np.float64), P * r)
        Sb2 += np.roll(out["bcol"].astype(np.float64), P * r)

    S1 = S1.reshape(-1) + C1
    S2 = S2.reshape(-1) + C2
    Sb1 = Sb1.reshape(-1)

    # exact diagonal corrections from the same fp8 data the device used
    r1f = r1_8.astype(np.float64)
    r2f = r2_8.astype(np.float64)
    q1 = CEXP * (r1f * r1f).sum(1)
    q2 = CEXP * (r2f * r2f).sum(1)
    v5 = (h1.astype(np.float64) * h2.astype(np.float64)).sum(1) / TAU

    d1 = S1 + Sb1 - np.exp(q1)
    d2 = S2 + Sb2 - np.exp(q2)
    loss = 0.5 * (np.log(d1) + np.log(d2)) - v5
    return np.float32(loss.sum())
